# revision 1
# baseline (speedup 1.0000x reference)
"""Trainium2 Bass kernel for BaseFisheyeLSSTransform (BEV pooling).

Strategy (output-sharded uniform SPMD over 8 NeuronCores):
- Host (cheap, index-only math): replicate the reference voxelization on
  jax-cpu fp32 to get each kept point's (batch, x-row, cy, 1/count). Points
  are grouped per output x-row, ordered by source memory index, merged into
  multi-row spans, and encoded as indirect-DMA descriptors (class-2 spans
  of <=2 rows, class-8 spans of 3..8 rows).
- Device: each core owns a balanced subset of x-rows of one batch. Per
  instruction: one indirect DMA gathers 128 descriptors from x[b]
  ([566400, 80] fp32) into SBUF [128, L*80]. Per column-block l a single
  fused DVE op builds M = (iota360 == vid)*invcnt, and partition-sliced
  matmuls accumulate PSUM[row] += X_l^T @ M_l ([80, 360] per x-row).
  Closed rows are copied to an SBUF slab and flushed to DRAM [80, NSLOTS*360].
- The instruction structure is identical on all cores (SPMD); all per-core
  variation is carried in data slabs (descriptor starts, vid, invcnt).
- Host assembles the final [2, 80, 360, 360] from the 8 slabs (pure unshard:
  each x-row is produced by exactly one core; empty rows are zeros).
"""
import sys

sys.path.insert(0, "/opt/trn_rl_repo")

import numpy as np

B, N, C = 2, 4, 80
FH, FW, D = 40, 60, 59
NX, NY = 360, 360
PB = N * D * FH * FW  # 566400 rows per batch slice of x
GAP_TOL = 2
P = 128
QUANT = 64
FLUSH_WINDOWS = 16


# ---------------------------------------------------------------- schedule


def _geometry(camera2lidar_rots, camera2lidar_trans):
    import jax
    import jax.numpy as jnp

    cpu = jax.devices("cpu")[0]
    with jax.default_device(cpu):
        DX = jnp.array([0.3, 0.3, 8.0], dtype=jnp.float32)
        ORIGIN = jnp.array([-54.0, -54.0, -5.0], dtype=jnp.float32)
        ds = jnp.arange(1.0, 60.0, 1.0, dtype=jnp.float32)
        az = jnp.linspace(-1.92, 1.92, FW, dtype=jnp.float32)
        el = jnp.linspace(-0.61, 0.61, FH, dtype=jnp.float32)
        d_, e_, a_ = ds[:, None, None], el[None, :, None], az[None, None, :]
        xs = d_ * jnp.cos(e_) * jnp.sin(a_)
        ys = jnp.broadcast_to(d_ * jnp.sin(e_), (D, FH, FW))
        zs = d_ * jnp.cos(e_) * jnp.cos(a_)
        fr = jnp.stack([xs, ys, zs], axis=-1)
        geom = jnp.einsum("bnij,dhwj->bndhwi", camera2lidar_rots, fr)
        geom = geom + camera2lidar_trans[:, :, None, None, None, :]
        coords = np.asarray(((geom - ORIGIN) / DX).astype(jnp.int32))
    kept = (
        (coords[..., 0] >= 0) & (coords[..., 0] < NX)
        & (coords[..., 1] >= 0) & (coords[..., 1] < NY)
        & (coords[..., 2] >= 0) & (coords[..., 2] < 1)
    )
    return coords, kept


def _build_rows(coords, kept):
    rows = {}
    for b in range(B):
        k = kept[b].reshape(-1)
        cx = coords[b, ..., 0].reshape(-1)
        cy = coords[b, ..., 1].reshape(-1)
        pts = np.flatnonzero(k)
        lin = cx[pts].astype(np.int64) * NY + cy[pts]
        cnt = np.bincount(lin, minlength=NX * NY)
        order = np.lexsort((pts, cx[pts]))
        sp = pts[order]
        sx = cx[pts][order]
        sy = cy[pts][order]
        w = (1.0 / np.maximum(cnt[lin[order]], 1)).astype(np.float32)
        new = np.ones(sp.size, bool)
        new[1:] = (np.diff(sx) != 0) | (np.diff(sp) > (GAP_TOL + 1))
        starts = np.flatnonzero(new)
        ends = np.append(starts[1:], sp.size)
        for s, e in zip(starts, ends):
            key = (b, int(sx[s]))
            if key not in rows:
                rows[key] = {2: [], 8: []}
            lane = {int(sp[i]): (int(sy[i]), float(w[i])) for i in range(s, e)}
            lo, hi = int(sp[s]), int(sp[e - 1])
            base = lo
            while base <= hi:
                span = hi - base + 1
                L = 2 if span <= 2 else 8
                start = max(0, min(base, PB - L))
                vids, ws = [], []
                for l in range(L):
                    r = start + l
                    if r in lane and r >= base:
                        vids.append(lane[r][0])
                        ws.append(lane[r][1])
                    else:
                        vids.append(-1)
                        ws.append(0.0)
                rows[key][L].append((start, vids, ws))
                base = start + L
    return rows


def _assign_cores(rows):
    cores = [[] for _ in range(8)]
    load = [0] * 8
    for b in range(B):
        keys = [k for k in rows if k[0] == b]
        keys.sort(key=lambda k: -(len(rows[k][2]) + len(rows[k][8])))
        for k in keys:
            cost = len(rows[k][2]) + len(rows[k][8])
            ci = min(range(4 * b, 4 * b + 4), key=lambda i: load[i])
            cores[ci].append(k)
            load[ci] += cost
    return cores, load


def _ceil(a, b):
    return -(-a // b)


def _build_uniform_schedule(rows, cores):
    core_rows = []
    NW = 0
    for ci in range(8):
        ks = sorted(cores[ci], key=lambda k: -(len(rows[k][2]) + len(rows[k][8])))
        core_rows.append(ks)
        NW = max(NW, len(ks))

    q2 = np.zeros(NW, np.int64)
    q8 = np.zeros(NW, np.int64)
    for ci in range(8):
        for w, key in enumerate(core_rows[ci]):
            q2[w] = max(q2[w], _ceil(len(rows[key][2]), QUANT))
            q8[w] = max(q8[w], _ceil(len(rows[key][8]), QUANT))

    def stream_instrs(qcounts):
        # Lane masking on lhsT makes any slice legal; pack maximally.
        NQ_PER_INSTR = P // QUANT
        instrs = []
        cur = []
        used = 0
        for w in range(NW):
            need = int(qcounts[w])
            while need > 0:
                take = min(NQ_PER_INSTR - used, need)
                cur.append((w, used * QUANT, (used + take) * QUANT))
                used += take
                need -= take
                if used == NQ_PER_INSTR:
                    instrs.append(cur)
                    cur = []
                    used = 0
        if cur:
            instrs.append(cur)
        return instrs

    i2 = stream_instrs(q2)
    i8 = stream_instrs(q8)
    tagged = [(min(t[0] for t in ins), 0, j, 2, ins) for j, ins in enumerate(i2)]
    tagged += [(min(t[0] for t in ins), 1, j, 8, ins) for j, ins in enumerate(i8)]
    tagged.sort(key=lambda t: (t[0], t[1], t[2]))

    struct = []
    cb0 = 0
    first_seen = {}
    last_seen = {}
    for ii, (_, _, _, cls, ins) in enumerate(tagged):
        tasks = [[l, lo, hi, w, False, False] for (w, lo, hi) in ins
                 for l in range(cls)]
        for (w, lo, hi) in ins:
            if w not in first_seen:
                first_seen[w] = ii
            last_seen[w] = ii
        struct.append(dict(cls=cls, cb0=cb0, tasks=tasks, copies_after=[]))
        cb0 += cls
    NCB = cb0
    NINSTR = len(struct)

    started = set()
    for rec in struct:
        for t in rec["tasks"]:
            if t[3] not in started:
                started.add(t[3])
                t[4] = True
    for w, ii in last_seen.items():
        rec = struct[ii]
        lastj = max(j for j, t in enumerate(rec["tasks"]) if t[3] == w)
        rec["tasks"][lastj][5] = True
    for rec in struct:
        rec["tasks"] = [tuple(t) for t in rec["tasks"]]
    for w, ii in last_seen.items():
        struct[ii]["copies_after"].append(w)
    NSLOTS = NW
    nblocks = _ceil(NSLOTS, FLUSH_WINDOWS)
    for k in range(nblocks):
        ws = [w for w in range(k * FLUSH_WINDOWS,
                               min((k + 1) * FLUSH_WINDOWS, NSLOTS))
              if w in last_seen]
        pos = max(last_seen[w] for w in ws) if ws else 0
        struct[pos].setdefault("flushes", []).append(k)

    per_core = []
    for ci in range(8):
        desc = np.zeros((P, NINSTR), np.int32)
        vid = np.full((P, NCB), -1.0, np.float32)
        invpc = np.zeros((P, NCB), np.float32)
        slot_rows = [None] * NSLOTS
        for w, key in enumerate(core_rows[ci]):
            slot_rows[w] = key
        cursor = {}
        for ii, rec in enumerate(struct):
            cls = rec["cls"]
            seen = set()
            for (l, lo, hi, w, st, sp_) in rec["tasks"]:
                if (w, lo) in seen:
                    continue
                seen.add((w, lo))
                if w >= len(core_rows[ci]):
                    continue
                key = core_rows[ci][w]
                dlist = rows[key][cls]
                cur = cursor.get((cls, w), 0)
                chunk = dlist[cur : cur + (hi - lo)]
                cursor[(cls, w)] = cur + (hi - lo)
                for j, (start, vids, ws_) in enumerate(chunk):
                    p_ = lo + j
                    desc[p_, ii] = start
                    for l2 in range(cls):
                        vid[p_, rec["cb0"] + l2] = vids[l2]
                        invpc[p_, rec["cb0"] + l2] = ws_[l2]
        per_core.append(dict(desc=desc, vid=vid, invpc=invpc,
                             slot_rows=slot_rows))

    return dict(struct=struct, NSLOTS=NSLOTS, NINSTR=NINSTR, NCB=NCB,
                per_core=per_core, nblocks=nblocks)


def build_schedule(camera2lidar_rots, camera2lidar_trans):
    coords, kept = _geometry(camera2lidar_rots, camera2lidar_trans)
    rows = _build_rows(coords, kept)
    cores, load = _assign_cores(rows)
    sched = _build_uniform_schedule(rows, cores)
    sched["load"] = load
    return sched


# ---------------------------------------------------------------- device


def mask_bank():
    combos = [(lo, hi) for lo in (0, 32, 64, 96) for hi in (32, 64, 96, 128)
              if lo < hi and not (lo == 0 and hi == 128)]
    mb = np.zeros((P, len(combos)), np.float32)
    for i, (lo, hi) in enumerate(combos):
        mb[lo:hi, i] = 1.0
    return mb


def build_program(sched):
    import concourse.bacc as bacc
    import concourse.bass as bass
    import concourse.mybir as mybir
    import concourse.tile as tile

    f32, i32 = mybir.dt.float32, mybir.dt.int32
    NINSTR, NCB, NSLOTS = sched["NINSTR"], sched["NCB"], sched["NSLOTS"]

    MASK_COMBOS = [(lo, hi) for lo in (0, 32, 64, 96) for hi in (32, 64, 96, 128)
                   if lo < hi and not (lo == 0 and hi == 128)]

    nc = bacc.Bacc(None)
    xb = nc.declare_dram_parameter("xb", [PB, C], f32, isOutput=False)
    maskb_d = nc.declare_dram_parameter("maskb", [P, len(MASK_COMBOS)], f32,
                                        isOutput=False)
    desc_d = nc.declare_dram_parameter("desc", [P, NINSTR], i32, isOutput=False)
    vid_d = nc.declare_dram_parameter("vid", [P, NCB], f32, isOutput=False)
    invpc_d = nc.declare_dram_parameter("invpc", [P, NCB], f32, isOutput=False)
    iota_d = nc.declare_dram_parameter("iota", [P, NY], f32, isOutput=False)
    out_d = nc.declare_dram_parameter("out", [C, NSLOTS * NY], f32,
                                      isOutput=True)

    with tile.TileContext(nc) as tc:
        with (
            tc.tile_pool(name="const", bufs=1) as cpool,
            tc.tile_pool(name="g2", bufs=8) as g2pool,
            tc.tile_pool(name="g8", bufs=4) as g8pool,
            tc.tile_pool(name="m", bufs=8) as mpool,
            tc.tile_pool(name="psum", bufs=8, space="PSUM") as ppool,
            tc.tile_pool(name="slab", bufs=3) as slabpool,
        ):
            desc_t = cpool.tile([P, NINSTR], i32)
            vid_t = cpool.tile([P, NCB], f32)
            invpc_t = cpool.tile([P, NCB], f32)
            iota_t = cpool.tile([P, NY], f32)
            maskb_t = cpool.tile([P, len(MASK_COMBOS)], f32)
            nc.sync.dma_start(out=maskb_t[:], in_=maskb_d[:])
            masks = {c: maskb_t[:, i : i + 1] for i, c in enumerate(MASK_COMBOS)}
            nc.sync.dma_start(out=desc_t[:], in_=desc_d[:])
            nc.sync.dma_start(out=vid_t[:], in_=vid_d[:])
            nc.sync.dma_start(out=invpc_t[:], in_=invpc_d[:])
            nc.sync.dma_start(out=iota_t[:], in_=iota_d[:])

            wtiles = {}
            slabs = {}
            for ii, rec in enumerate(sched["struct"]):
                L = rec["cls"]
                pool = g2pool if L == 2 else g8pool
                g = pool.tile([P, L * C], f32, tag=f"g{L}")
                nc.gpsimd.indirect_dma_start(
                    out=g[:],
                    out_offset=None,
                    in_=xb[:],
                    in_offset=bass.IndirectOffsetOnAxis(
                        ap=desc_t[:, ii : ii + 1], axis=0
                    ),
                )
                Ms = {}
                for l in range(L):
                    col = rec["cb0"] + l
                    M = mpool.tile([P, NY], f32, tag="m")
                    # M = (iota == vid) * invcnt, fused on DVE
                    nc.vector.tensor_scalar(
                        out=M[:],
                        in0=iota_t[:],
                        scalar1=vid_t[:, col : col + 1],
                        scalar2=invpc_t[:, col : col + 1],
                        op0=mybir.AluOpType.is_equal,
                        op1=mybir.AluOpType.mult,
                    )
                    Ms[l] = M
                for (l, lo, hi, w, st, sp_) in rec["tasks"]:
                    if st:
                        wtiles[w] = ppool.tile([C, NY], f32, tag="w", name=f"w{w}")
                    if lo == 0 and hi == 128:
                        lhs = g[:, l * C : (l + 1) * C]
                    else:
                        # full-K matmul with lanes outside [lo,hi) zeroed on
                        # the 80-wide lhsT (partition-sliced matmuls that
                        # accumulate are an HW/compiler hazard).
                        xm = mpool.tile([P, C], f32, tag="xm", name="xm")
                        nc.vector.tensor_scalar_mul(
                            xm[:], g[:, l * C : (l + 1) * C], masks[(lo, hi)]
                        )
                        lhs = xm[:]
                    nc.tensor.matmul(
                        wtiles[w][:],
                        lhs,
                        Ms[l][:],
                        start=st,
                        stop=sp_,
                        skip_group_check=True,
                    )
                for w in rec["copies_after"]:
                    blk = w // FLUSH_WINDOWS
                    if blk not in slabs:
                        slabs[blk] = slabpool.tile(
                            [C, FLUSH_WINDOWS * NY], f32, tag="slab",
                            name=f"slab{blk}",
                        )
                    off = w % FLUSH_WINDOWS
                    nc.vector.tensor_copy(
                        slabs[blk][:, off * NY : (off + 1) * NY],
                        wtiles.pop(w)[:],
                    )
                for blk in rec.get("flushes", []):
                    w0 = blk * FLUSH_WINDOWS
                    w1 = min(w0 + FLUSH_WINDOWS, NSLOTS)
                    nc.sync.dma_start(
                        out=out_d[:, w0 * NY : w1 * NY],
                        in_=slabs.pop(blk)[:, : (w1 - w0) * NY],
                    )
    nc.compile()
    return nc


def run_on_device(sched, x):
    from concourse.bass_utils import run_bass_kernel_spmd

    nc = build_program(sched)
    iota = np.broadcast_to(
        np.arange(NY, dtype=np.float32)[None, :], (P, NY)
    ).copy()
    maskb = mask_bank()
    in_maps = []
    for ci in range(8):
        b = 0 if ci < 4 else 1
        pc = sched["per_core"][ci]
        in_maps.append(
            {
                "xb": np.ascontiguousarray(x[b].reshape(PB, C)),
                "desc": pc["desc"],
                "vid": pc["vid"],
                "invpc": pc["invpc"],
                "iota": iota,
                "maskb": maskb,
            }
        )
    res = run_bass_kernel_spmd(nc, in_maps, list(range(8)))
    return [res.results[ci]["out"] for ci in range(8)]


def assemble(slabs, sched):
    out = np.zeros((B, C, NX, NY), np.float32)
    for ci in range(8):
        pc = sched["per_core"][ci]
        slab = slabs[ci]
        for s, key in enumerate(pc["slot_rows"]):
            if key is None:
                continue
            b, xrow = key
            out[b, :, xrow, :] = slab[:, s * NY : (s + 1) * NY]
    return out


def kernel(x, camera2lidar_rots, camera2lidar_trans):
    x = np.asarray(x, dtype=np.float32)
    rots = np.asarray(camera2lidar_rots, dtype=np.float32)
    trans = np.asarray(camera2lidar_trans, dtype=np.float32)
    sched = build_schedule(rots, trans)
    slabs = run_on_device(sched, x)
    return assemble(slabs, sched)



# revision 2
# speedup vs baseline: 1.5372x; 1.5372x over previous
"""Trainium2 Bass kernel for BaseFisheyeLSSTransform (BEV mean-pooling).

Architecture (SPMD over 8 cores, uniform program, slot-space output):
- Host (index-only math from the small rots/trans inputs): voxelize; kept
  points form memory runs (consecutive rows, gap<=GAP). Runs are split into
  L=8-row span descriptors, descriptors sorted by cell (y-major voxel id),
  chunked contiguously across 4 cores per batch (balanced +-1), and packed
  into instructions of <=P descriptors and <=W distinct cells.
- Device per instruction: indirect-DMA gather (P spans x L rows x 80 f32
  from x[b], cast bf16 on the fly) -> 8 lanes; per lane DVE builds
  M[p, j] = (iota[j] == vid[p]) * invcnt[p]  (bf16, iota/vid shifted by
  -W/2 so bf16 holds them exactly), PE accumulates psum[80, W] += g_l^T M.
  Psum is copied (bf16) into a slab, flushed contiguously to DRAM in
  FLUSH_WIN-instruction blocks.
- Host: add slab columns into the [NX*NY] grid per the (core, instr, col)
  -> cell tables (partial sums across instructions add up; invcnt is
  folded into M so this is already the mean), then lay out [B, C, NX, NY].
"""
import sys

sys.path.insert(0, "/opt/trn_rl_repo")

import numpy as np

B, N, C = 2, 4, 80
FH, FW, D = 40, 60, 59
NX, NY = 360, 360
PB = N * D * FH * FW  # 566400 rows per batch slice of x
GAP = 8    # max dumped-point gap merged into a span
L = 8      # rows per gather descriptor
P = 128    # descriptors per gather instruction
W = 512    # psum window cap (max distinct cells per instruction)
FLUSH_WIN = 8  # instructions per slab flush


def _geometry(camera2lidar_rots, camera2lidar_trans):
    import jax
    import jax.numpy as jnp

    cpu = jax.devices("cpu")[0]
    with jax.default_device(cpu):
        DX = jnp.array([0.3, 0.3, 8.0], dtype=jnp.float32)
        ORIGIN = jnp.array([-54.0, -54.0, -5.0], dtype=jnp.float32)
        ds = jnp.arange(1.0, 60.0, 1.0, dtype=jnp.float32)
        az = jnp.linspace(-1.92, 1.92, FW, dtype=jnp.float32)
        el = jnp.linspace(-0.61, 0.61, FH, dtype=jnp.float32)
        d_, e_, a_ = ds[:, None, None], el[None, :, None], az[None, None, :]
        xs = d_ * jnp.cos(e_) * jnp.sin(a_)
        ys = jnp.broadcast_to(d_ * jnp.sin(e_), (D, FH, FW))
        zs = d_ * jnp.cos(e_) * jnp.cos(a_)
        fr = jnp.stack([xs, ys, zs], axis=-1)
        geom = jnp.einsum("bnij,dhwj->bndhwi", camera2lidar_rots, fr)
        geom = geom + camera2lidar_trans[:, :, None, None, None, :]
        coords = np.asarray(((geom - ORIGIN) / DX).astype(jnp.int32))
    kept = (
        (coords[..., 0] >= 0) & (coords[..., 0] < NX)
        & (coords[..., 1] >= 0) & (coords[..., 1] < NY)
        & (coords[..., 2] >= 0) & (coords[..., 2] < 1)
    )
    return coords, kept


def _batch_descs(pts, lin2, invc_pt):
    """Split kept points (ascending memory rows `pts`, cell id `lin2`,
    per-point weight `invc_pt`) into L-row span descriptors sorted by cell.

    Returns desc arrays: start[nd], vid_cell[nd, L] (int64 cell id, -1 =
    dead lane), wgt[nd, L] (f32).
    """
    brk = np.ones(pts.size, bool)
    brk[1:] = np.diff(pts) > (GAP + 1)
    run_id = np.cumsum(brk) - 1
    run_lo = pts[np.flatnonzero(brk)]  # first row of each run
    # descriptor index within run: offset//L
    off = pts - run_lo[run_id]
    desc_of_pt_in_run = off // L
    lane = off % L
    # global desc id: (run_id, desc_of_pt_in_run) -> unique
    nd_per_run = np.zeros(run_lo.size, np.int64)
    np.maximum.at(nd_per_run, run_id, desc_of_pt_in_run + 1)
    desc_base = np.concatenate([[0], np.cumsum(nd_per_run)[:-1]])
    did = desc_base[run_id] + desc_of_pt_in_run
    nd = int(nd_per_run.sum())
    start = np.zeros(nd, np.int64)
    start[did] = run_lo[run_id] + desc_of_pt_in_run * L  # same for all pts in desc
    # clamp starts so start+L <= PB (descs never read OOB; lanes shift)
    start = np.minimum(start, PB - L)
    lane = pts - start[did]
    assert (lane >= 0).all() and (lane < L).all()
    vid_cell = np.full((nd, L), -1, np.int64)
    wgt = np.zeros((nd, L), np.float32)
    vid_cell[did, lane] = lin2
    wgt[did, lane] = invc_pt
    # sort descs by first live cell id (cell locality for packing)
    firstcell = np.where(vid_cell >= 0, vid_cell, np.iinfo(np.int64).max).min(
        axis=1
    )
    order = np.argsort(firstcell, kind="stable")
    return start[order], vid_cell[order], wgt[order]


def _pack_instrs(start, vid_cell, wgt):
    """Greedy-pack descriptors (in given order) into instructions with
    <=P descs and <=W distinct cells. Returns list of
    (desc_slice, cells_array, vid_local[P, L], wgt[P, L], start[P])."""
    nd = start.shape[0]
    instrs = []
    i = 0
    while i < nd:
        cells = {}
        j = i
        while j < nd and j - i < P:
            newc = [c for c in vid_cell[j] if c >= 0 and c not in cells]
            if len(cells) + len(set(newc)) > W:
                break
            for c in newc:
                if c not in cells:
                    cells[c] = len(cells)
            j += 1
        cell_arr = np.fromiter(cells.keys(), np.int64, len(cells))
        k = j - i
        vloc = np.full((P, L), -1.0, np.float32)
        wg = np.zeros((P, L), np.float32)
        st = np.zeros(P, np.int64)
        for d in range(k):
            st[d] = start[i + d]
            for l in range(L):
                c = vid_cell[i + d, l]
                if c >= 0:
                    vloc[d, l] = cells[c]
                    wg[d, l] = wgt[i + d, l]
        instrs.append((cell_arr, vloc, wg, st))
        i = j
    return instrs


def build_schedule(camera2lidar_rots, camera2lidar_trans):
    coords, kept = _geometry(camera2lidar_rots, camera2lidar_trans)
    cores = []  # 8 entries: dict(b, instrs)
    for b in range(B):
        k = kept[b].reshape(-1)
        cx = coords[b, ..., 0].reshape(-1)
        cy = coords[b, ..., 1].reshape(-1)
        pts = np.flatnonzero(k)
        lin2 = cy[pts].astype(np.int64) * NX + cx[pts]
        cnt_full = np.bincount(lin2, minlength=NX * NY)
        invc_pt = (1.0 / cnt_full[lin2]).astype(np.float32)
        start, vid_cell, wgt = _batch_descs(pts, lin2, invc_pt)
        # pack the whole batch, then deal instructions to 4 cores
        # round-robin (keeps per-slot ncells profiles similar across cores)
        instrs = _pack_instrs(start, vid_cell, wgt)
        instrs.sort(key=lambda t: -t[0].size)  # align widths across deal
        for ci in range(4):
            cores.append(dict(b=b, instrs=instrs[ci::4]))

    NI = max(len(cr["instrs"]) for cr in cores)
    # per-slot shared widths: ncols[ii] = max ncells, quantized; nlan[ii]
    ncols = np.zeros(NI, np.int64)
    nlan = np.zeros(NI, np.int64)
    for cr in cores:
        for ii, (cell_arr, vloc, wg, st) in enumerate(cr["instrs"]):
            ncols[ii] = max(ncols[ii], cell_arr.size)
            live = np.flatnonzero((vloc >= 0).any(axis=0))
            if live.size:
                nlan[ii] = max(nlan[ii], live[-1] + 1)
    ncols = np.minimum(-(-ncols // 32) * 32, W)
    ncols = np.maximum(ncols, 64)
    nlan = np.maximum(nlan, 1)
    col_off = np.concatenate([[0], np.cumsum(ncols)])
    NCOL = int(col_off[-1])

    # device-side constant data per core
    per_core = []
    for cr in cores:
        desc = np.zeros((P, NI), np.int32)
        vid = np.full((P, NI * L), -1e30, np.float32)
        ivw = np.zeros((P, NI * L), np.float32)
        cell_tables = [np.zeros(0, np.int64)] * NI
        for ii, (cell_arr, vloc, wg, st) in enumerate(cr["instrs"]):
            desc[:, ii] = st
            # shift into bf16-exact range (global shift, iota matches)
            sh = W // 2
            vid[:, ii * L : (ii + 1) * L] = np.where(
                vloc >= 0, vloc - sh, -1e30
            )
            ivw[:, ii * L : (ii + 1) * L] = wg
            cell_tables[ii] = cell_arr
        per_core.append(
            dict(b=cr["b"], desc=desc, vid=vid, ivw=ivw, cells=cell_tables)
        )
    return dict(NI=NI, per_core=per_core, ncols=ncols, nlan=nlan,
                col_off=col_off, NCOL=NCOL)


# ---------------------------------------------------------------- simulator


def simulate(sched, x):
    """Pure-numpy replica of the device program (bf16 rounding included)."""
    import ml_dtypes

    bf16 = ml_dtypes.bfloat16
    NI = sched["NI"]
    ncols, nlan = sched["ncols"], sched["nlan"]
    out = np.zeros((B, NX * NY, C), np.float32)  # grid[lin2, c]
    for ci, pc in enumerate(sched["per_core"]):
        xb = x[pc["b"]].reshape(PB, C)
        for ii in range(NI):
            Wi, Li = int(ncols[ii]), int(nlan[ii])
            st = pc["desc"][:, ii]
            g = np.zeros((P, L * C), np.float32)
            for d in range(P):
                g[d] = xb[st[d] : st[d] + L].reshape(-1)
            g = g.astype(bf16)  # cast during gather
            vid = pc["vid"][:, ii * L : (ii + 1) * L]
            ivw = pc["ivw"][:, ii * L : (ii + 1) * L]
            iota = np.arange(Wi, dtype=np.float32) - W // 2
            psum = np.zeros((C, Wi), np.float32)
            for l in range(Li):
                M = (
                    (iota[None, :] == vid[:, l : l + 1]).astype(np.float32)
                    * ivw[:, l : l + 1].astype(np.float32)
                ).astype(bf16)
                psum += (
                    g[:, l * C : (l + 1) * C].astype(np.float32).T
                    @ M.astype(np.float32)
                )
            slab = psum.astype(bf16).astype(np.float32)  # copy cast
            if ii < len(pc["cells"]):
                cells = pc["cells"][ii]
                out[pc["b"], cells] += slab[:, : cells.size].T
    final = np.zeros((B, C, NX, NY), np.float32)
    grid = out.reshape(B, NY, NX, C)  # lin2 = cy*NX + cx
    final = grid.transpose(0, 3, 2, 1)  # [b, c, cx, cy]
    return np.ascontiguousarray(final)


# ---------------------------------------------------------------- device


def build_program(sched):
    import concourse.bacc as bacc
    import concourse.bass as bass
    import concourse.mybir as mybir
    import concourse.tile as tile

    f32 = mybir.dt.float32
    i32 = mybir.dt.int32
    bf16 = mybir.dt.bfloat16
    NI = sched["NI"]
    ncols, nlan = sched["ncols"], sched["nlan"]
    col_off, NCOL = sched["col_off"], sched["NCOL"]

    nc = bacc.Bacc(None)
    xb = nc.declare_dram_parameter("xb", [PB, C], f32, isOutput=False)
    desc_d = nc.declare_dram_parameter("desc", [P, NI], i32, isOutput=False)
    vid_d = nc.declare_dram_parameter("vid", [P, NI * L], f32, isOutput=False)
    ivw_d = nc.declare_dram_parameter("ivw", [P, NI * L], f32, isOutput=False)
    iota_d = nc.declare_dram_parameter("iota", [P, W], f32, isOutput=False)
    out_d = nc.declare_dram_parameter("out", [C, NCOL], bf16, isOutput=True)

    with tile.TileContext(nc) as tc:
        with (
            tc.tile_pool(name="const", bufs=1) as cpool,
            tc.tile_pool(name="g", bufs=6) as gpool,
            tc.tile_pool(name="m", bufs=16) as mpool,
            tc.tile_pool(name="psum", bufs=8, space="PSUM") as ppool,
            tc.tile_pool(name="slab", bufs=2) as spool,
        ):
            desc_t = cpool.tile([P, NI], i32)
            vid_t = cpool.tile([P, NI * L], f32)
            ivw_t = cpool.tile([P, NI * L], f32)
            iota_t = cpool.tile([P, W], f32)
            nc.sync.dma_start(out=desc_t[:], in_=desc_d[:])
            nc.sync.dma_start(out=vid_t[:], in_=vid_d[:])
            nc.sync.dma_start(out=ivw_t[:], in_=ivw_d[:])
            nc.sync.dma_start(out=iota_t[:], in_=iota_d[:])

            slab = None
            slab_base = 0
            for ii in range(NI):
                Wi, Li = int(ncols[ii]), int(nlan[ii])
                if ii % FLUSH_WIN == 0:
                    i0 = ii
                    i1 = min(ii + FLUSH_WIN, NI)
                    slab_base = int(col_off[i0])
                    span = int(col_off[i1]) - slab_base
                    slab = spool.tile([C, span], bf16, tag="slab")
                g = gpool.tile([P, L * C], bf16, tag="g")
                nc.gpsimd.indirect_dma_start(
                    out=g[:],
                    out_offset=None,
                    in_=xb[:],
                    in_offset=bass.IndirectOffsetOnAxis(
                        ap=desc_t[:, ii : ii + 1], axis=0
                    ),
                )
                pt = ppool.tile([C, Wi], f32, tag="w")
                for l in range(Li):
                    col = ii * L + l
                    M = mpool.tile([P, Wi], bf16, tag="m")
                    nc.vector.tensor_scalar(
                        out=M[:],
                        in0=iota_t[:, :Wi],
                        scalar1=vid_t[:, col : col + 1],
                        scalar2=ivw_t[:, col : col + 1],
                        op0=mybir.AluOpType.is_equal,
                        op1=mybir.AluOpType.mult,
                    )
                    nc.tensor.matmul(
                        pt[:],
                        g[:, l * C : (l + 1) * C],
                        M[:],
                        start=(l == 0),
                        stop=(l == Li - 1),
                    )
                off = int(col_off[ii]) - slab_base
                nc.vector.tensor_copy(slab[:, off : off + Wi], pt[:])
                if ii % FLUSH_WIN == FLUSH_WIN - 1 or ii == NI - 1:
                    i0 = (ii // FLUSH_WIN) * FLUSH_WIN
                    span = int(col_off[ii + 1]) - int(col_off[i0])
                    nc.sync.dma_start(
                        out=out_d[:, int(col_off[i0]) : int(col_off[ii + 1])],
                        in_=slab[:, :span],
                    )
    nc.compile()
    return nc


def _iota_np():
    return np.broadcast_to(
        (np.arange(W, dtype=np.float32) - W // 2)[None, :], (P, W)
    ).copy()


def make_in_maps(sched, x):
    import ml_dtypes

    bf = ml_dtypes.bfloat16
    iota = _iota_np()
    in_maps = []
    for pc in sched["per_core"]:
        in_maps.append(
            {
                "xb": np.ascontiguousarray(x[pc["b"]].reshape(PB, C)),
                "desc": pc["desc"],
                "vid": pc["vid"],
                "ivw": pc["ivw"],
                "iota": iota,
            }
        )
    return in_maps


def assemble(slabs, sched):
    grid = np.zeros((B, NX * NY, C), np.float32)
    for ci, pc in enumerate(sched["per_core"]):
        slab = np.asarray(slabs[ci]).astype(np.float32)  # [C, NI*W]
        co = sched["col_off"]
        for ii, cells in enumerate(pc["cells"]):
            if cells.size:
                o = int(co[ii])
                grid[pc["b"], cells] += slab[:, o : o + cells.size].T
    g = grid.reshape(B, NY, NX, C)
    return np.ascontiguousarray(g.transpose(0, 3, 2, 1))


def kernel(x, camera2lidar_rots, camera2lidar_trans):
    from concourse.bass_utils import run_bass_kernel_spmd

    x = np.asarray(x, dtype=np.float32)
    rots = np.asarray(camera2lidar_rots, dtype=np.float32)
    trans = np.asarray(camera2lidar_trans, dtype=np.float32)
    sched = build_schedule(rots, trans)
    nc = build_program(sched)
    in_maps = make_in_maps(sched, x)
    res = run_bass_kernel_spmd(nc, in_maps, list(range(8)))
    slabs = [res.results[ci]["out"] for ci in range(8)]
    return assemble(slabs, sched)


# revision 3
# speedup vs baseline: 1.7579x; 1.1435x over previous
"""Trainium2 Bass kernel for BaseFisheyeLSSTransform (BEV mean-pooling).

Architecture (SPMD over 8 cores, uniform program, slot-space output):
- Host (index-only math from the small rots/trans inputs): voxelize; kept
  points form memory runs (consecutive rows, gap<=GAP). Runs are split into
  L=8-row span descriptors, descriptors sorted by cell (y-major voxel id),
  chunked contiguously across 4 cores per batch (balanced +-1), and packed
  into instructions of <=P descriptors and <=W distinct cells.
- Device per instruction: indirect-DMA gather (P spans x L rows x 80 f32
  from x[b], cast bf16 on the fly) -> 8 lanes; per lane DVE builds
  M[p, j] = (iota[j] == vid[p]) * invcnt[p]  (bf16, iota/vid shifted by
  -W/2 so bf16 holds them exactly), PE accumulates psum[80, W] += g_l^T M.
  Psum is copied (bf16) into a slab, flushed contiguously to DRAM in
  FLUSH_WIN-instruction blocks.
- Host: add slab columns into the [NX*NY] grid per the (core, instr, col)
  -> cell tables (partial sums across instructions add up; invcnt is
  folded into M so this is already the mean), then lay out [B, C, NX, NY].
"""
import sys

sys.path.insert(0, "/opt/trn_rl_repo")

import numpy as np

B, N, C = 2, 4, 80
FH, FW, D = 40, 60, 59
NX, NY = 360, 360
PB = N * D * FH * FW  # 566400 rows per batch slice of x
GAP = 16   # max dumped-point gap merged into a span
L = 8      # rows per gather descriptor
P = 128    # descriptors per gather instruction
W = 512    # psum window cap (max distinct cells per instruction)
FLUSH_WIN = 8  # instructions per slab flush


def _geometry(camera2lidar_rots, camera2lidar_trans):
    import jax
    import jax.numpy as jnp

    cpu = jax.devices("cpu")[0]
    with jax.default_device(cpu):
        DX = jnp.array([0.3, 0.3, 8.0], dtype=jnp.float32)
        ORIGIN = jnp.array([-54.0, -54.0, -5.0], dtype=jnp.float32)
        ds = jnp.arange(1.0, 60.0, 1.0, dtype=jnp.float32)
        az = jnp.linspace(-1.92, 1.92, FW, dtype=jnp.float32)
        el = jnp.linspace(-0.61, 0.61, FH, dtype=jnp.float32)
        d_, e_, a_ = ds[:, None, None], el[None, :, None], az[None, None, :]
        xs = d_ * jnp.cos(e_) * jnp.sin(a_)
        ys = jnp.broadcast_to(d_ * jnp.sin(e_), (D, FH, FW))
        zs = d_ * jnp.cos(e_) * jnp.cos(a_)
        fr = jnp.stack([xs, ys, zs], axis=-1)
        geom = jnp.einsum("bnij,dhwj->bndhwi", camera2lidar_rots, fr)
        geom = geom + camera2lidar_trans[:, :, None, None, None, :]
        coords = np.asarray(((geom - ORIGIN) / DX).astype(jnp.int32))
    kept = (
        (coords[..., 0] >= 0) & (coords[..., 0] < NX)
        & (coords[..., 1] >= 0) & (coords[..., 1] < NY)
        & (coords[..., 2] >= 0) & (coords[..., 2] < 1)
    )
    return coords, kept


def _batch_descs(pts, lin2, invc_pt):
    """Split kept points (ascending memory rows `pts`, cell id `lin2`,
    per-point weight `invc_pt`) into L-row span descriptors sorted by cell.

    Returns desc arrays: start[nd], vid_cell[nd, L] (int64 cell id, -1 =
    dead lane), wgt[nd, L] (f32).
    """
    brk = np.ones(pts.size, bool)
    brk[1:] = np.diff(pts) > (GAP + 1)
    run_id = np.cumsum(brk) - 1
    run_lo = pts[np.flatnonzero(brk)]  # first row of each run
    # descriptor index within run: offset//L
    off = pts - run_lo[run_id]
    desc_of_pt_in_run = off // L
    lane = off % L
    # global desc id: (run_id, desc_of_pt_in_run) -> unique
    nd_per_run = np.zeros(run_lo.size, np.int64)
    np.maximum.at(nd_per_run, run_id, desc_of_pt_in_run + 1)
    desc_base = np.concatenate([[0], np.cumsum(nd_per_run)[:-1]])
    did = desc_base[run_id] + desc_of_pt_in_run
    nd = int(nd_per_run.sum())
    start = np.zeros(nd, np.int64)
    start[did] = run_lo[run_id] + desc_of_pt_in_run * L  # same for all pts in desc
    # clamp starts so start+L <= PB (descs never read OOB; lanes shift)
    start = np.minimum(start, PB - L)
    lane = pts - start[did]
    assert (lane >= 0).all() and (lane < L).all()
    vid_cell = np.full((nd, L), -1, np.int64)
    wgt = np.zeros((nd, L), np.float32)
    vid_cell[did, lane] = lin2
    wgt[did, lane] = invc_pt
    # sort descs by first live cell id (cell locality for packing)
    firstcell = np.where(vid_cell >= 0, vid_cell, np.iinfo(np.int64).max).min(
        axis=1
    )
    order = np.argsort(firstcell, kind="stable")
    return start[order], vid_cell[order], wgt[order]


def _pack_instrs(start, vid_cell, wgt):
    """Greedy-pack descriptors (in given order) into instructions with
    <=P descs and <=W distinct cells. Returns list of
    (desc_slice, cells_array, vid_local[P, L], wgt[P, L], start[P])."""
    nd = start.shape[0]
    instrs = []
    i = 0
    while i < nd:
        cells = {}
        j = i
        while j < nd and j - i < P:
            newc = [c for c in vid_cell[j] if c >= 0 and c not in cells]
            if len(cells) + len(set(newc)) > W:
                break
            for c in newc:
                if c not in cells:
                    cells[c] = len(cells)
            j += 1
        cell_arr = np.fromiter(cells.keys(), np.int64, len(cells))
        k = j - i
        vloc = np.full((P, L), -1.0, np.float32)
        wg = np.zeros((P, L), np.float32)
        st = np.zeros(P, np.int64)
        for d in range(k):
            st[d] = start[i + d]
            for l in range(L):
                c = vid_cell[i + d, l]
                if c >= 0:
                    vloc[d, l] = cells[c]
                    wg[d, l] = wgt[i + d, l]
        instrs.append((cell_arr, vloc, wg, st))
        i = j
    return instrs


def build_schedule(camera2lidar_rots, camera2lidar_trans):
    coords, kept = _geometry(camera2lidar_rots, camera2lidar_trans)
    cores = []  # 8 entries: dict(b, instrs)
    for b in range(B):
        k = kept[b].reshape(-1)
        cx = coords[b, ..., 0].reshape(-1)
        cy = coords[b, ..., 1].reshape(-1)
        pts = np.flatnonzero(k)
        lin2 = cy[pts].astype(np.int64) * NX + cx[pts]
        cnt_full = np.bincount(lin2, minlength=NX * NY)
        invc_pt = (1.0 / cnt_full[lin2]).astype(np.float32)
        start, vid_cell, wgt = _batch_descs(pts, lin2, invc_pt)
        # pack the whole batch, then deal instructions to 4 cores
        # round-robin (keeps per-slot ncells profiles similar across cores)
        instrs = _pack_instrs(start, vid_cell, wgt)
        instrs.sort(key=lambda t: -t[0].size)  # align widths across deal
        for ci in range(4):
            cores.append(dict(b=b, instrs=instrs[ci::4]))

    NI = max(len(cr["instrs"]) for cr in cores)
    # per-slot shared widths: ncols[ii] = max ncells, quantized; nlan[ii]
    ncols = np.zeros(NI, np.int64)
    nlan = np.zeros(NI, np.int64)
    for cr in cores:
        for ii, (cell_arr, vloc, wg, st) in enumerate(cr["instrs"]):
            ncols[ii] = max(ncols[ii], cell_arr.size)
            live = np.flatnonzero((vloc >= 0).any(axis=0))
            if live.size:
                nlan[ii] = max(nlan[ii], live[-1] + 1)
    ncols = np.minimum(-(-ncols // 32) * 32, W)
    ncols = np.maximum(ncols, 64)
    nlan = np.maximum(nlan, 1)
    col_off = np.concatenate([[0], np.cumsum(ncols)])
    NCOL = int(col_off[-1])

    # device-side constant data per core
    per_core = []
    for cr in cores:
        desc = np.zeros((P, NI), np.int32)
        vid = np.full((P, NI * L), -1e30, np.float32)
        ivw = np.zeros((P, NI * L), np.float32)
        cell_tables = [np.zeros(0, np.int64)] * NI
        for ii, (cell_arr, vloc, wg, st) in enumerate(cr["instrs"]):
            desc[:, ii] = st
            # shift into bf16-exact range (global shift, iota matches)
            sh = W // 2
            vid[:, ii * L : (ii + 1) * L] = np.where(
                vloc >= 0, vloc - sh, -1e30
            )
            ivw[:, ii * L : (ii + 1) * L] = wg
            cell_tables[ii] = cell_arr
        per_core.append(
            dict(b=cr["b"], desc=desc, vid=vid, ivw=ivw, cells=cell_tables)
        )
    return dict(NI=NI, per_core=per_core, ncols=ncols, nlan=nlan,
                col_off=col_off, NCOL=NCOL)


# ---------------------------------------------------------------- simulator


def simulate(sched, x):
    """Pure-numpy replica of the device program (bf16 rounding included)."""
    import ml_dtypes

    bf16 = ml_dtypes.bfloat16
    NI = sched["NI"]
    ncols, nlan = sched["ncols"], sched["nlan"]
    out = np.zeros((B, NX * NY, C), np.float32)  # grid[lin2, c]
    for ci, pc in enumerate(sched["per_core"]):
        xb = x[pc["b"]].reshape(PB, C)
        for ii in range(NI):
            Wi, Li = int(ncols[ii]), int(nlan[ii])
            st = pc["desc"][:, ii]
            g = np.zeros((P, L * C), np.float32)
            for d in range(P):
                g[d] = xb[st[d] : st[d] + L].reshape(-1)
            g = g.astype(bf16)  # cast during gather
            vid = pc["vid"][:, ii * L : (ii + 1) * L]
            ivw = pc["ivw"][:, ii * L : (ii + 1) * L]
            iota = np.arange(Wi, dtype=np.float32) - W // 2
            psum = np.zeros((C, Wi), np.float32)
            for l in range(Li):
                M = (
                    (iota[None, :] == vid[:, l : l + 1]).astype(np.float32)
                    * ivw[:, l : l + 1].astype(np.float32)
                ).astype(bf16)
                psum += (
                    g[:, l * C : (l + 1) * C].astype(np.float32).T
                    @ M.astype(np.float32)
                )
            slab = psum.astype(bf16).astype(np.float32)  # copy cast
            if ii < len(pc["cells"]):
                cells = pc["cells"][ii]
                out[pc["b"], cells] += slab[:, : cells.size].T
    final = np.zeros((B, C, NX, NY), np.float32)
    grid = out.reshape(B, NY, NX, C)  # lin2 = cy*NX + cx
    final = grid.transpose(0, 3, 2, 1)  # [b, c, cx, cy]
    return np.ascontiguousarray(final)


# ---------------------------------------------------------------- device


def build_program(sched):
    import concourse.bacc as bacc
    import concourse.bass as bass
    import concourse.mybir as mybir
    import concourse.tile as tile

    f32 = mybir.dt.float32
    i32 = mybir.dt.int32
    bf16 = mybir.dt.bfloat16
    NI = sched["NI"]
    ncols, nlan = sched["ncols"], sched["nlan"]
    col_off, NCOL = sched["col_off"], sched["NCOL"]

    nc = bacc.Bacc(None)
    xb = nc.declare_dram_parameter("xb", [PB, C], f32, isOutput=False)
    desc_d = nc.declare_dram_parameter("desc", [P, NI], i32, isOutput=False)
    vid_d = nc.declare_dram_parameter("vid", [P, NI * L], f32, isOutput=False)
    ivw_d = nc.declare_dram_parameter("ivw", [P, NI * L], f32, isOutput=False)
    iota_d = nc.declare_dram_parameter("iota", [P, W], f32, isOutput=False)
    out_d = nc.declare_dram_parameter("out", [C, NCOL], bf16, isOutput=True)

    with tile.TileContext(nc) as tc:
        with (
            tc.tile_pool(name="const", bufs=1) as cpool,
            tc.tile_pool(name="g", bufs=6) as gpool,
            tc.tile_pool(name="m", bufs=16) as mpool,
            tc.tile_pool(name="psum", bufs=8, space="PSUM") as ppool,
            tc.tile_pool(name="slab", bufs=2) as spool,
        ):
            desc_t = cpool.tile([P, NI], i32)
            vid_t = cpool.tile([P, NI * L], f32)
            ivw_t = cpool.tile([P, NI * L], f32)
            iota_t = cpool.tile([P, W], f32)
            nc.sync.dma_start(out=desc_t[:], in_=desc_d[:])
            nc.sync.dma_start(out=vid_t[:], in_=vid_d[:])
            nc.sync.dma_start(out=ivw_t[:], in_=ivw_d[:])
            nc.sync.dma_start(out=iota_t[:], in_=iota_d[:])

            slab = None
            slab_base = 0
            for ii in range(NI):
                Wi, Li = int(ncols[ii]), int(nlan[ii])
                if ii % FLUSH_WIN == 0:
                    i0 = ii
                    i1 = min(ii + FLUSH_WIN, NI)
                    slab_base = int(col_off[i0])
                    span = int(col_off[i1]) - slab_base
                    slab = spool.tile([C, span], bf16, tag="slab")
                g = gpool.tile([P, L * C], bf16, tag="g")
                nc.gpsimd.indirect_dma_start(
                    out=g[:],
                    out_offset=None,
                    in_=xb[:],
                    in_offset=bass.IndirectOffsetOnAxis(
                        ap=desc_t[:, ii : ii + 1], axis=0
                    ),
                )
                pt = ppool.tile([C, Wi], f32, tag="w")
                for l in range(Li):
                    col = ii * L + l
                    M = mpool.tile([P, Wi], bf16, tag="m")
                    nc.vector.tensor_scalar(
                        out=M[:],
                        in0=iota_t[:, :Wi],
                        scalar1=vid_t[:, col : col + 1],
                        scalar2=ivw_t[:, col : col + 1],
                        op0=mybir.AluOpType.is_equal,
                        op1=mybir.AluOpType.mult,
                    )
                    nc.tensor.matmul(
                        pt[:],
                        g[:, l * C : (l + 1) * C],
                        M[:],
                        start=(l == 0),
                        stop=(l == Li - 1),
                    )
                off = int(col_off[ii]) - slab_base
                nc.scalar.copy(slab[:, off : off + Wi], pt[:])
                if ii % FLUSH_WIN == FLUSH_WIN - 1 or ii == NI - 1:
                    i0 = (ii // FLUSH_WIN) * FLUSH_WIN
                    span = int(col_off[ii + 1]) - int(col_off[i0])
                    nc.sync.dma_start(
                        out=out_d[:, int(col_off[i0]) : int(col_off[ii + 1])],
                        in_=slab[:, :span],
                    )
    nc.compile()
    return nc


def _iota_np():
    return np.broadcast_to(
        (np.arange(W, dtype=np.float32) - W // 2)[None, :], (P, W)
    ).copy()


def make_in_maps(sched, x):
    import ml_dtypes

    bf = ml_dtypes.bfloat16
    iota = _iota_np()
    in_maps = []
    for pc in sched["per_core"]:
        in_maps.append(
            {
                "xb": np.ascontiguousarray(x[pc["b"]].reshape(PB, C)),
                "desc": pc["desc"],
                "vid": pc["vid"],
                "ivw": pc["ivw"],
                "iota": iota,
            }
        )
    return in_maps


def assemble(slabs, sched):
    grid = np.zeros((B, NX * NY, C), np.float32)
    for ci, pc in enumerate(sched["per_core"]):
        slab = np.asarray(slabs[ci]).astype(np.float32)  # [C, NI*W]
        co = sched["col_off"]
        for ii, cells in enumerate(pc["cells"]):
            if cells.size:
                o = int(co[ii])
                grid[pc["b"], cells] += slab[:, o : o + cells.size].T
    g = grid.reshape(B, NY, NX, C)
    return np.ascontiguousarray(g.transpose(0, 3, 2, 1))


def kernel(x, camera2lidar_rots, camera2lidar_trans):
    from concourse.bass_utils import run_bass_kernel_spmd

    x = np.asarray(x, dtype=np.float32)
    rots = np.asarray(camera2lidar_rots, dtype=np.float32)
    trans = np.asarray(camera2lidar_trans, dtype=np.float32)
    sched = build_schedule(rots, trans)
    nc = build_program(sched)
    in_maps = make_in_maps(sched, x)
    res = run_bass_kernel_spmd(nc, in_maps, list(range(8)))
    slabs = [res.results[ci]["out"] for ci in range(8)]
    return assemble(slabs, sched)
